# revision 2
# baseline (speedup 1.0000x reference)
"""Locally-connected 1D conv (per-output-position weights) on 8 trn2 NeuronCores.

out[b,d,o] = relu(sum_{c,k} x[b,c,o+k] * w[d,c,o,k] + bias[d])
B=16, C=32, D=32, K=16, O=8176 (IN=8192).

Strategy: shard the output dimension O across 8 cores (1022 each). w (535MB)
dominates traffic; the host pre-packs each core's w shard into a matmul-ready,
DMA-friendly layout (partition-outermost, contiguous per partition) and builds
a small 4x-im2col of x so every SBUF access pattern on device is a plain 2D
slice. Per output position o: 4 accumulating matmuls with contraction
(khat4, c32)=128; w-chunk [128x32] is the stationary operand (streams through
LDWEIGHTS), the x-window [128x16] is the moving operand; PSUM holds [d32 x b16]
per o, 32 o's per PSUM bank. ScalarE evacuates with fused bias+ReLU (bias is
per-partition because d lands on PSUM partitions).
"""

import numpy as np

import concourse.bacc as bacc
import concourse.mybir as mybir
from concourse import bass_utils
from concourse.bass import ds
from concourse.tile import TileContext

B, C, D, K, O, IN = 16, 32, 32, 16, 8176, 8192
NCORES = 8
OSH = O // NCORES  # 1022 outputs per core
SLEN = OSH + (K - 4)  # 1034 window-start positions (s = o + 4q, q<4)
XWIN = OSH + K - 1  # 1037 x columns needed per core
PT = 32  # outputs per PSUM tile (32*16=512 f32 = one bank)
OT = 32  # outputs per w2 DMA block

_CACHE = {}


def _build():
    if "nc" in _CACHE:
        return _CACHE["nc"]
    nc = bacc.Bacc("TRN2", target_bir_lowering=False, debug=False)
    f32 = mybir.dt.float32
    w2 = nc.dram_tensor("w2", (128, OSH * 4 * 32), f32, kind="ExternalInput")
    s_in = nc.dram_tensor("s", (128, SLEN * B), f32, kind="ExternalInput")
    bias = nc.dram_tensor("bias", (D, 1), f32, kind="ExternalInput")
    out = nc.dram_tensor("out", (D, OSH * B), f32, kind="ExternalOutput")

    nblk = (OSH + OT - 1) // OT
    with TileContext(nc) as tc:
        with (
            tc.tile_pool(name="const", bufs=1) as cpool,
            tc.tile_pool(name="wpool", bufs=3) as wpool,
            tc.tile_pool(name="opool", bufs=3) as opool,
            tc.tile_pool(name="psum", bufs=4, space="PSUM") as ppool,
        ):
            s_tile = cpool.tile([128, SLEN * B], f32)
            nc.sync.dma_start(out=s_tile[:, :], in_=s_in[:, :])
            b_tile = cpool.tile([D, 1], f32)
            nc.sync.dma_start(out=b_tile[:, :], in_=bias[:, :])

            for blk in range(nblk):
                o0 = blk * OT
                no = min(OT, OSH - o0)
                wt = wpool.tile([128, OT * 128], f32, tag="wt")
                nc.sync.dma_start(
                    out=wt[:, : no * 128], in_=w2[:, ds(o0 * 128, no * 128)]
                )
                psum = ppool.tile([D, PT * B], f32, tag="ps")
                for ol in range(no):
                    o = o0 + ol
                    for q in range(4):
                        nc.tensor.matmul(
                            psum[:, ds(ol * B, B)],
                            wt[:, ds(ol * 128 + q * 32, 32)],
                            s_tile[:, ds((o + 4 * q) * B, B)],
                            start=(q == 0),
                            stop=(q == 3),
                        )
                ot = opool.tile([D, PT * B], f32, tag="ot")
                nc.scalar.activation(
                    ot[:, : no * B],
                    psum[:, : no * B],
                    mybir.ActivationFunctionType.Relu,
                    bias=b_tile[:, :],
                    scale=1.0,
                )
                nc.sync.dma_start(out=out[:, ds(o0 * B, no * B)], in_=ot[:, : no * B])

    nc.compile()
    _CACHE["nc"] = nc
    return nc


def _pack_core(x, w, b, i):
    o0 = i * OSH
    # w2[p=(khat*32+c)][o][q][d] = w[d, c, o0+o, 4q+khat]
    wi = w[:, :, o0 : o0 + OSH, :]  # (D, C, OSH, K)
    a = wi.transpose(3, 1, 2, 0)  # (K, C, OSH, D) = [k][c][o][d]
    a = a.reshape(4, 4, C, OSH, D)  # [q][khat][c][o][d]
    a = a.transpose(1, 2, 3, 0, 4)  # [khat][c][o][q][d]
    w2 = np.ascontiguousarray(a.reshape(128, OSH * 4 * D), dtype=np.float32)
    # s[p=(khat*32+c)][s][b] = x[b, c, o0+s+khat]
    xs = x[:, :, o0 : o0 + XWIN]  # (B, C, XWIN)
    sa = np.stack([xs[:, :, kh : kh + SLEN] for kh in range(4)], axis=0)
    sa = sa.transpose(0, 2, 3, 1)  # (4, C, SLEN, B)
    s_host = np.ascontiguousarray(sa.reshape(128, SLEN * B), dtype=np.float32)
    bias = np.ascontiguousarray(b.reshape(D, 1), dtype=np.float32)
    return {"w2": w2, "s": s_host, "bias": bias}


def kernel(x, w, b, _results_hook=None):
    x = np.asarray(x, dtype=np.float32)
    w = np.asarray(w, dtype=np.float32)
    b = np.asarray(b, dtype=np.float32)
    nc = _build()
    in_maps = [_pack_core(x, w, b, i) for i in range(NCORES)]
    import os

    trace = bool(int(os.environ.get("KTRACE", "0")))
    res = bass_utils.run_bass_kernel_spmd(
        nc, in_maps, core_ids=list(range(NCORES)), trace=trace
    )
    if _results_hook is not None:
        _results_hook(res)
    parts = []
    for i in range(NCORES):
        oi = res.results[i]["out"].reshape(D, OSH, B)
        parts.append(oi.transpose(2, 0, 1))  # (B, D, OSH)
    return np.ascontiguousarray(np.concatenate(parts, axis=2))


# revision 6
# speedup vs baseline: 22.5186x; 22.5186x over previous
"""Locally-connected 1D conv (per-output-position weights) on 8 trn2 NeuronCores.

out[b,d,o] = relu(sum_{c,k} x[b,c,o+k] * w[d,c,o,k] + bias[d])
B=16, C=32, D=32, K=16, O=8176 (IN=8192).

Strategy: shard the output dimension O across 8 cores (1022 each). w (535MB)
dominates traffic; the host pre-packs each core's w shard into a matmul-ready,
DMA-friendly layout (partition-outermost, contiguous per partition) and builds
a small 4x-im2col of x so every SBUF access pattern on device is a plain 2D
slice. Per output position o: 4 accumulating matmuls with contraction
(khat4, c32)=128; w-chunk [128x32] is the stationary operand (streams through
LDWEIGHTS), the x-window [128x16] is the moving operand; PSUM holds [d32 x b16]
per o, 32 o's per PSUM bank. ScalarE evacuates with fused bias+ReLU (bias is
per-partition because d lands on PSUM partitions).
"""

import numpy as np

import concourse.bacc as bacc
import concourse.mybir as mybir
from concourse import bass_utils
from concourse.bass import ds
from concourse.tile import TileContext

B, C, D, K, O, IN = 16, 32, 32, 16, 8176, 8192
NCORES = 8
OSH = O // NCORES  # 1022 outputs per core
SLEN = OSH + (K - 4)  # 1034 window-start positions (s = o + 4q, q<4)
XWIN = OSH + K - 1  # 1037 x columns needed per core
PT = 32  # outputs per PSUM tile (32*16=512 f32 = one bank)
OT = 64  # outputs per w2 DMA block (4MB DMAs)

_CACHE = {}


def _build():
    if "nc" in _CACHE:
        return _CACHE["nc"]
    nc = bacc.Bacc("TRN2", target_bir_lowering=False, debug=False)
    f32 = mybir.dt.float32
    w2 = nc.dram_tensor("w2", (128, OSH * 4 * 32), f32, kind="ExternalInput")
    s_in = nc.dram_tensor("s", (128, SLEN * B), f32, kind="ExternalInput")
    bias = nc.dram_tensor("bias", (D, 1), f32, kind="ExternalInput")
    out = nc.dram_tensor("out", (D, OSH * B), f32, kind="ExternalOutput")

    nblk = (OSH + OT - 1) // OT
    with TileContext(nc) as tc:
        with (
            tc.tile_pool(name="const", bufs=1) as cpool,
            tc.tile_pool(name="wpool", bufs=3) as wpool,
            tc.tile_pool(name="opool", bufs=3) as opool,
            tc.tile_pool(name="psum", bufs=4, space="PSUM") as ppool,
        ):
            s_tile = cpool.tile([128, SLEN * B], f32)
            # split the S load so the first matmuls unblock early; use the
            # ACT HWDGE queue so it doesn't FIFO-block w2 loads on sync
            SCH = 4
            cs = (SLEN * B + SCH - 1) // SCH
            for c0 in range(0, SLEN * B, cs):
                cn = min(cs, SLEN * B - c0)
                nc.scalar.dma_start(
                    out=s_tile[:, ds(c0, cn)], in_=s_in[:, ds(c0, cn)]
                )
            b_tile = cpool.tile([D, 1], f32)
            nc.scalar.dma_start(out=b_tile[:, :], in_=bias[:, :])

            for blk in range(nblk):
                o0 = blk * OT
                no = min(OT, OSH - o0)
                wt = wpool.tile([128, OT * 128], f32, tag="wt")
                nc.sync.dma_start(
                    out=wt[:, : no * 128], in_=w2[:, ds(o0 * 128, no * 128)]
                )
                ot = opool.tile([D, OT * B], f32, tag="ot")
                for p0 in range(0, no, PT):
                    np_ = min(PT, no - p0)
                    psum = ppool.tile([D, PT * B], f32, tag="ps")
                    for ol in range(p0, p0 + np_):
                        o = o0 + ol
                        for q in range(4):
                            nc.tensor.matmul(
                                psum[:, ds((ol - p0) * B, B)],
                                wt[:, ds(ol * 128 + q * 32, 32)],
                                s_tile[:, ds((o + 4 * q) * B, B)],
                                start=(q == 0),
                                stop=(q == 3),
                            )
                    nc.scalar.activation(
                        ot[:, ds(p0 * B, np_ * B)],
                        psum[:, : np_ * B],
                        mybir.ActivationFunctionType.Relu,
                        bias=b_tile[:, :],
                        scale=1.0,
                    )
                nc.scalar.dma_start(
                    out=out[:, ds(o0 * B, no * B)], in_=ot[:, : no * B]
                )

    nc.compile()
    _CACHE["nc"] = nc
    return nc


def _pack_core(x, w, b, i):
    o0 = i * OSH
    # w2[p=(khat*32+c)][o][q][d] = w[d, c, o0+o, 4q+khat]
    wi = w[:, :, o0 : o0 + OSH, :]  # (D, C, OSH, K)
    a = wi.transpose(3, 1, 2, 0)  # (K, C, OSH, D) = [k][c][o][d]
    a = a.reshape(4, 4, C, OSH, D)  # [q][khat][c][o][d]
    a = a.transpose(1, 2, 3, 0, 4)  # [khat][c][o][q][d]
    w2 = np.ascontiguousarray(a.reshape(128, OSH * 4 * D), dtype=np.float32)
    # s[p=(khat*32+c)][s][b] = x[b, c, o0+s+khat]
    xs = x[:, :, o0 : o0 + XWIN]  # (B, C, XWIN)
    sa = np.stack([xs[:, :, kh : kh + SLEN] for kh in range(4)], axis=0)
    sa = sa.transpose(0, 2, 3, 1)  # (4, C, SLEN, B)
    s_host = np.ascontiguousarray(sa.reshape(128, SLEN * B), dtype=np.float32)
    bias = np.ascontiguousarray(b.reshape(D, 1), dtype=np.float32)
    return {"w2": w2, "s": s_host, "bias": bias}


def kernel(x, w, b, _results_hook=None):
    x = np.asarray(x, dtype=np.float32)
    w = np.asarray(w, dtype=np.float32)
    b = np.asarray(b, dtype=np.float32)
    nc = _build()
    in_maps = [_pack_core(x, w, b, i) for i in range(NCORES)]
    import os

    trace = bool(int(os.environ.get("KTRACE", "0")))
    res = bass_utils.run_bass_kernel_spmd(
        nc, in_maps, core_ids=list(range(NCORES)), trace=trace
    )
    if _results_hook is not None:
        _results_hook(res)
    parts = []
    for i in range(NCORES):
        oi = res.results[i]["out"].reshape(D, OSH, B)
        parts.append(oi.transpose(2, 0, 1))  # (B, D, OSH)
    return np.ascontiguousarray(np.concatenate(parts, axis=2))


# revision 9
# speedup vs baseline: 22.6047x; 1.0038x over previous
"""Locally-connected 1D conv (per-output-position weights) on 8 trn2 NeuronCores.

out[b,d,o] = relu(sum_{c,k} x[b,c,o+k] * w[d,c,o,k] + bias[d])
B=16, C=32, D=32, K=16, O=8176 (IN=8192).

Strategy: shard the output dimension O across 8 cores (1022 each). w (535MB)
dominates traffic; the host pre-packs each core's w shard into a matmul-ready,
DMA-friendly layout (partition-outermost, contiguous per partition) and builds
a small 4x-im2col of x so every SBUF access pattern on device is a plain 2D
slice. Per output position o: 4 accumulating matmuls with contraction
(khat4, c32)=128; w-chunk [128x32] is the stationary operand (streams through
LDWEIGHTS), the x-window [128x16] is the moving operand; PSUM holds [d32 x b16]
per o, 32 o's per PSUM bank. ScalarE evacuates with fused bias+ReLU (bias is
per-partition because d lands on PSUM partitions).
"""

import numpy as np

import concourse.bacc as bacc
import concourse.mybir as mybir
from concourse import bass_utils
from concourse.bass import ds
from concourse.tile import TileContext

B, C, D, K, O, IN = 16, 32, 32, 16, 8176, 8192
NCORES = 8
OSH = O // NCORES  # 1022 outputs per core
SLEN = OSH + (K - 4)  # 1034 window-start positions (s = o + 4q, q<4)
XWIN = OSH + K - 1  # 1037 x columns needed per core
PT = 32  # outputs per PSUM tile (32*16=512 f32 = one bank)
OT = 64  # outputs per w2 DMA block (4MB DMAs)

_CACHE = {}


def _build():
    if "nc" in _CACHE:
        return _CACHE["nc"]
    nc = bacc.Bacc("TRN2", target_bir_lowering=False, debug=False)
    f32 = mybir.dt.float32
    w2 = nc.dram_tensor("w2", (128, OSH * 4 * 32), f32, kind="ExternalInput")
    s_in = nc.dram_tensor("s", (128, SLEN * B), f32, kind="ExternalInput")
    bias = nc.dram_tensor("bias", (D, 1), f32, kind="ExternalInput")
    out = nc.dram_tensor("out", (D, OSH * B), f32, kind="ExternalOutput")

    nblk = (OSH + OT - 1) // OT
    with TileContext(nc) as tc:
        with (
            tc.tile_pool(name="const", bufs=1) as cpool,
            tc.tile_pool(name="wpool", bufs=3) as wpool,
            tc.tile_pool(name="opool", bufs=3) as opool,
            tc.tile_pool(name="psum", bufs=8, space="PSUM") as ppool,
        ):
            s_tile = cpool.tile([128, SLEN * B], f32)
            # split the S load so the first matmuls unblock early; use the
            # ACT HWDGE queue so it doesn't FIFO-block w2 loads on sync
            SCH = 8
            cs = (SLEN * B + SCH - 1) // SCH
            for c0 in range(0, SLEN * B, cs):
                cn = min(cs, SLEN * B - c0)
                nc.scalar.dma_start(
                    out=s_tile[:, ds(c0, cn)], in_=s_in[:, ds(c0, cn)]
                )
            b_tile = cpool.tile([D, 1], f32)
            nc.scalar.dma_start(out=b_tile[:, :], in_=bias[:, :])

            # small first block so the PE starts after ~1MB of w2 instead of
            # 4MB; remainder in OT-sized blocks (ragged tail handled below)
            sizes = [16]
            while sum(sizes) < OSH:
                sizes.append(min(OT, OSH - sum(sizes)))
            offs = [sum(sizes[:i]) for i in range(len(sizes))]
            for o0, no in zip(offs, sizes):
                wt = wpool.tile([128, OT * 128], f32, tag="wt")
                nc.sync.dma_start(
                    out=wt[:, : no * 128], in_=w2[:, ds(o0 * 128, no * 128)]
                )
                ot = opool.tile([D, OT * B], f32, tag="ot")
                for p0 in range(0, no, PT):
                    np_ = min(PT, no - p0)
                    psum = ppool.tile([D, PT * B], f32, tag="ps")
                    for ol in range(p0, p0 + np_):
                        o = o0 + ol
                        for q in range(4):
                            nc.tensor.matmul(
                                psum[:, ds((ol - p0) * B, B)],
                                wt[:, ds(ol * 128 + q * 32, 32)],
                                s_tile[:, ds((o + 4 * q) * B, B)],
                                start=(q == 0),
                                stop=(q == 3),
                            )
                    nc.scalar.activation(
                        ot[:, ds(p0 * B, np_ * B)],
                        psum[:, : np_ * B],
                        mybir.ActivationFunctionType.Relu,
                        bias=b_tile[:, :],
                        scale=1.0,
                    )
                nc.scalar.dma_start(
                    out=out[:, ds(o0 * B, no * B)], in_=ot[:, : no * B]
                )

    nc.compile()
    _CACHE["nc"] = nc
    return nc


def _pack_core(x, w, b, i):
    o0 = i * OSH
    # w2[p=(khat*32+c)][o][q][d] = w[d, c, o0+o, 4q+khat]
    wi = w[:, :, o0 : o0 + OSH, :]  # (D, C, OSH, K)
    a = wi.transpose(3, 1, 2, 0)  # (K, C, OSH, D) = [k][c][o][d]
    a = a.reshape(4, 4, C, OSH, D)  # [q][khat][c][o][d]
    a = a.transpose(1, 2, 3, 0, 4)  # [khat][c][o][q][d]
    w2 = np.ascontiguousarray(a.reshape(128, OSH * 4 * D), dtype=np.float32)
    # s[p=(khat*32+c)][s][b] = x[b, c, o0+s+khat]
    xs = x[:, :, o0 : o0 + XWIN]  # (B, C, XWIN)
    sa = np.stack([xs[:, :, kh : kh + SLEN] for kh in range(4)], axis=0)
    sa = sa.transpose(0, 2, 3, 1)  # (4, C, SLEN, B)
    s_host = np.ascontiguousarray(sa.reshape(128, SLEN * B), dtype=np.float32)
    bias = np.ascontiguousarray(b.reshape(D, 1), dtype=np.float32)
    return {"w2": w2, "s": s_host, "bias": bias}


def kernel(x, w, b, _results_hook=None):
    x = np.asarray(x, dtype=np.float32)
    w = np.asarray(w, dtype=np.float32)
    b = np.asarray(b, dtype=np.float32)
    nc = _build()
    in_maps = [_pack_core(x, w, b, i) for i in range(NCORES)]
    import os

    trace = bool(int(os.environ.get("KTRACE", "0")))
    res = bass_utils.run_bass_kernel_spmd(
        nc, in_maps, core_ids=list(range(NCORES)), trace=trace
    )
    if _results_hook is not None:
        _results_hook(res)
    parts = []
    for i in range(NCORES):
        oi = res.results[i]["out"].reshape(D, OSH, B)
        parts.append(oi.transpose(2, 0, 1))  # (B, D, OSH)
    return np.ascontiguousarray(np.concatenate(parts, axis=2))
